# revision 13
# baseline (speedup 1.0000x reference)
"""Trainium2 Bass kernel for DAWN MoE routing block.

Problem (fixed shapes, fp32):
  x [8, 4096, 1024], importance [8, 4096], W_proj [1024, 64], b_proj [64],
  neuron_emb [7936, 64].
  h = x @ W_proj + b_proj                     [B, S, 64]
  all_logits = h @ normalize(neuron_emb).T    [B, S, 7936]
  per type-slice (qk 2048 | v 1024 | rel 512 | val 256 | know 4096):
    pref = softmax(logits_slice, axis=-1)
    w    = einsum('s,sn->n', importance, pref)
    top-k / sparsify per slice.

Key observations baked into this kernel:
  * The knowledge slice (last 4096 neurons) never reaches an output, so only
    the first 3840 logit columns are computed.
  * Outputs only need w[b, n] (a [8, 3840] array) from the device; the tiny
    top-k / sort / sparsify tail runs on host.
  * Data-parallel over batch: core i handles batch i (B == n_cores == 8).

Per-core device pipeline (hT = (x @ W + b).T computed once):
  pre-pass:  PE-transpose x tiles -> x^T, h^T = W^T @ x^T (PSUM accum),
             bias added on the PSUM->SBUF copy.
  main loop over 32 s-chunks of 128:
    logits chunk [128, 3840] via PE (lhsT = hT slice, rhs = embU^T),
    exp on ScalarE reading PSUM directly, with fused per-row accumulation
    giving the softmax denominators,
    scales = importance / denom (DVE), scattered into a mostly-zero [128, 48]
    stationary buffer Z,
    w accumulation: 8 matmuls per chunk with lhsT = an 8-wide sliding window
    of Z (so chunk j's scale column lands in output row j), all 256 matmuls
    accumulating into a single PSUM bank [8, 512].
"""

import os
import sys

sys.path.insert(0, "/opt/trn_rl_repo")

import numpy as np

# ---- hardcoded problem dims ----
B = 8
S = 4096
D_MODEL = 1024
D_SPACE = 64
N_QK, N_V, N_REL, N_VAL = 2048, 1024, 512, 256
NTOT = N_QK + N_V + N_REL + N_VAL  # 3840
TOPK_QK, TOPK_V, TOPK_REL, TOPK_VAL = 64, 32, 16, 3
N_CORES = 8
SCHUNK = 128
N_SCHUNKS = S // SCHUNK  # 32
N_NCHUNKS = 8  # neuron chunks of 512 (last one only 256 valid)
NCHUNK_SIZES = [512] * 7 + [256]
# Z layout: scale column for slice t sits at col ZCOL[t]; window for neuron
# chunk j starts at ZWIN[j] so that the scale lands at local column j.
ZCOL = [3, 15, 27, 39]
SLICE_OF_CHUNK = [0, 0, 0, 0, 1, 1, 2, 3]
ZWIN = [ZCOL[SLICE_OF_CHUNK[j]] - j for j in range(8)]  # [3,2,1,0,11,10,21,32]
ZW = 48

# matmul input mode: "f32r" (full-rate fp32), "bf16", or "f32" (4x slow, exact)
MM_MODE = os.environ.get("KERNEL_MM_MODE", "f32r")
TRACE = bool(int(os.environ.get("KERNEL_TRACE", "0")))

_compiled = {}


def _build(mode):
    from contextlib import ExitStack

    import concourse.bacc as bacc
    import concourse.tile as tile
    from concourse import mybir
    from concourse.masks import make_identity

    f32 = mybir.dt.float32
    bf16 = mybir.dt.bfloat16
    f32r = mybir.dt.float32r
    # dtype used for SBUF matmul operand tiles; fp32r tiles must be written
    # by instructions with fp32r output dtype (producer-side rounding).
    op_dt = {"f32r": f32r, "bf16": bf16, "f32": f32}[mode]

    def mm_ap(ap):
        return ap

    nc = bacc.Bacc("TRN2", target_bir_lowering=False, debug=False,
                   num_devices=N_CORES)

    x_d = nc.dram_tensor("x", [S, D_MODEL], f32, kind="ExternalInput").ap()
    imp_d = nc.dram_tensor("imp", [SCHUNK, N_SCHUNKS], f32,
                           kind="ExternalInput").ap()
    w_d = nc.dram_tensor("w_proj", [D_MODEL, D_SPACE], f32,
                         kind="ExternalInput").ap()
    b_d = nc.dram_tensor("b_proj", [D_SPACE, 1], f32, kind="ExternalInput").ap()
    embt_d = nc.dram_tensor("embt", [D_SPACE, NTOT],
                            bf16 if mode == "bf16" else f32,
                            kind="ExternalInput").ap()
    wout_d = nc.dram_tensor("wout", [N_NCHUNKS, 512], f32,
                            kind="ExternalOutput").ap()

    with tile.TileContext(nc) as tc:
        with (
            tc.tile_pool(name="const", bufs=1) as const_pool,
            tc.tile_pool(name="ht", bufs=1) as ht_pool,
        ):
            # ---- constants ----
            ident = const_pool.tile([128, 128], f32)
            make_identity(nc, ident)
            wt_raw = const_pool.tile([128, 8, D_SPACE], f32)
            nc.sync.dma_start(out=wt_raw,
                              in_=w_d.rearrange("(kc p) m -> p kc m", p=128))
            if mode == "f32":
                wt = wt_raw
            else:
                wt = const_pool.tile([128, 8, D_SPACE], op_dt, tag="wt_c")
                nc.vector.tensor_copy(out=wt, in_=wt_raw)
            if mode == "bf16":
                embt = const_pool.tile([D_SPACE, NTOT], op_dt)
                nc.sync.dma_start(out=embt, in_=embt_d)
            else:
                embt_raw = const_pool.tile([D_SPACE, NTOT], f32)
                nc.sync.dma_start(out=embt_raw, in_=embt_d)
                if mode == "f32":
                    embt = embt_raw
                else:
                    embt = const_pool.tile([D_SPACE, NTOT], op_dt, tag="embt_c")
                    nc.vector.tensor_copy(out=embt, in_=embt_raw)
            imp2d = const_pool.tile([SCHUNK, N_SCHUNKS], f32)
            nc.sync.dma_start(out=imp2d, in_=imp_d)
            bproj = const_pool.tile([D_SPACE, 1], f32)
            nc.sync.dma_start(out=bproj, in_=b_d)

            # hT: [64, S] stationary operand for the logits matmuls
            ht = ht_pool.tile([D_SPACE, S], op_dt)

            # ---- pre-pass: hT = W^T @ x^T + b ----
            SG = 512  # s-group width
            pre_stack = ExitStack()
            xg_pool = pre_stack.enter_context(tc.tile_pool(name="xg", bufs=2))
            xt_pool = pre_stack.enter_context(tc.tile_pool(name="xt", bufs=2))
            psum_t_pool = pre_stack.enter_context(
                tc.tile_pool(name="psum_t", bufs=2, space="PSUM"))
            psum_h_pool = pre_stack.enter_context(
                tc.tile_pool(name="psum_h", bufs=2, space="PSUM"))
            for g in range(S // SG):
                xg = xg_pool.tile([128, SG // 128, D_MODEL], f32)
                nc.sync.dma_start(
                    out=xg,
                    in_=x_d[g * SG:(g + 1) * SG, :].rearrange(
                        "(i p) d -> p i d", p=128),
                )
                xts = xt_pool.tile([128, 8, SG], op_dt)
                for kc in range(8):
                    pxt = psum_t_pool.tile([128, SG], f32)
                    for i in range(SG // 128):
                        nc.tensor.transpose(
                            pxt[:, i * 128:(i + 1) * 128],
                            xg[:, i, kc * 128:(kc + 1) * 128],
                            ident,
                        )
                    nc.vector.tensor_copy(out=xts[:, kc, :], in_=pxt)
                hps = psum_h_pool.tile([D_SPACE, SG], f32)
                for kc in range(8):
                    nc.tensor.matmul(
                        hps,
                        wt[:, kc, :],
                        xts[:, kc, :],
                        start=(kc == 0),
                        stop=(kc == 7),
                    )
                # bias + cast on PSUM->SBUF copy
                nc.vector.tensor_scalar_add(
                    out=ht[:, g * SG:(g + 1) * SG], in0=hps, scalar1=bproj)

            pre_stack.close()

            # ---- main pass ----
            main_stack = ExitStack()
            e_pool = main_stack.enter_context(tc.tile_pool(name="e", bufs=2))
            sc_pool = main_stack.enter_context(tc.tile_pool(name="sc", bufs=3))
            z_pool = main_stack.enter_context(tc.tile_pool(name="z", bufs=1))
            psum_lg_pool = main_stack.enter_context(
                tc.tile_pool(name="psum_lg", bufs=3, space="PSUM"))
            psum_w_pool = main_stack.enter_context(
                tc.tile_pool(name="psum_w", bufs=1, space="PSUM"))
            wacc = psum_w_pool.tile([N_NCHUNKS, 512], f32)
            # two alternating Z buffers; zero columns are written once and
            # never touched again (per-chunk writes hit only the scale cols)
            ztmp = z_pool.tile([128, ZW], f32, tag="ztmp")
            nc.vector.memset(ztmp, 0.0)
            zbufs = []
            for zi in range(2):
                zb = z_pool.tile([128, ZW], op_dt, tag=f"z{zi}")
                nc.vector.tensor_copy(out=zb, in_=ztmp)
                zbufs.append(zb)
            for c in range(N_SCHUNKS):
                hts = ht[:, c * SCHUNK:(c + 1) * SCHUNK]
                et = e_pool.tile([128, NTOT], op_dt)
                part = sc_pool.tile([128, 8], f32, tag="part")
                # logits + exp per psum tile
                # tiles: [qk0 1024][qk1 1024][v 1024][rel 512 | val 256]
                plans = [
                    (0, 1024, [0]),
                    (1024, 1024, [1]),
                    (2048, 1024, [2]),
                    (3072, 768, [3, 4]),
                ]
                acc_i = 0
                for lo, width, accs in plans:
                    lg = psum_lg_pool.tile([128, 1024], f32)
                    off = 0
                    while off < width:
                        n = min(512, width - off)
                        nc.tensor.matmul(
                            lg[:, off:off + n],
                            mm_ap(hts),
                            mm_ap(embt[:, lo + off:lo + off + n]),
                            start=True,
                            stop=True,
                        )
                        off += n
                    # exp + fused row-sum; one activation per slice segment
                    seg_off = 0
                    for a in accs:
                        seg_w = {0: 1024, 1: 1024, 2: 1024, 3: 512, 4: 256}[a]
                        nc.scalar.activation(
                            out=et[:, lo + seg_off:lo + seg_off + seg_w],
                            in_=lg[:, seg_off:seg_off + seg_w],
                            func=mybir.ActivationFunctionType.Exp,
                            accum_out=part[:, a:a + 1],
                        )
                        seg_off += seg_w
                # denominators: qk = part0+part1, v = part2, rel = part3,
                # val = part4
                d4 = sc_pool.tile([128, 4], f32, tag="d4")
                nc.vector.tensor_tensor(
                    out=d4[:, 0:1], in0=part[:, 0:1], in1=part[:, 1:2],
                    op=mybir.AluOpType.add)
                nc.vector.tensor_copy(out=d4[:, 1:4], in_=part[:, 2:5])
                r4 = sc_pool.tile([128, 4], f32, tag="r4")
                nc.vector.reciprocal(out=r4, in_=d4)
                z = zbufs[c % 2]
                nc.vector.tensor_scalar(
                    out=z[:, ZCOL[0]:ZCOL[3] + 1:12],
                    in0=r4,
                    scalar1=imp2d[:, c:c + 1],
                    scalar2=None,
                    op0=mybir.AluOpType.mult,
                )
                for j in range(N_NCHUNKS):
                    n = NCHUNK_SIZES[j]
                    nc.tensor.matmul(
                        wacc[:, 0:n],
                        mm_ap(z[:, ZWIN[j]:ZWIN[j] + 8]),
                        mm_ap(et[:, j * 512:j * 512 + n]),
                        start=(c == 0 and j == 0),
                        stop=(c == N_SCHUNKS - 1 and j == N_NCHUNKS - 1),
                        skip_group_check=True,
                    )
            wout_s = const_pool.tile([N_NCHUNKS, 512], f32, tag="wout")
            nc.vector.tensor_copy(out=wout_s, in_=wacc)
            nc.sync.dma_start(out=wout_d, in_=wout_s)
            main_stack.close()

    nc.compile()
    return nc


def _topk_sorted_idx(w, k):
    # jax.lax.top_k: descending value, ties -> lower index first
    idx = np.argsort(-w, axis=-1, kind="stable")[:, :k]
    return np.sort(idx, axis=-1).astype(np.int32)


def _topk_sparsify(w, k):
    idx = np.argsort(-w, axis=-1, kind="stable")[:, :k]
    out = np.zeros_like(w)
    rows = np.arange(w.shape[0])[:, None]
    out[rows, idx] = w[rows, idx]
    return out


def kernel(x, importance, W_proj, b_proj, neuron_emb):
    from concourse.bass_utils import run_bass_kernel_spmd

    x = np.ascontiguousarray(np.asarray(x, dtype=np.float32))
    importance = np.asarray(importance, dtype=np.float32)
    W_proj = np.ascontiguousarray(np.asarray(W_proj, dtype=np.float32))
    b_proj = np.asarray(b_proj, dtype=np.float32)
    neuron_emb = np.asarray(neuron_emb, dtype=np.float32)

    mode = MM_MODE
    if mode not in _compiled:
        _compiled[mode] = _build(mode)
    nc = _compiled[mode]

    # host-side prep (tiny): normalize emb, transpose, truncate
    embU = neuron_emb / np.linalg.norm(neuron_emb, axis=-1, keepdims=True)
    embT = np.ascontiguousarray(embU[:NTOT].T)  # [64, 3840] f32
    if mode == "bf16":
        import ml_dtypes
        embT = embT.astype(ml_dtypes.bfloat16)
    in_maps = []
    for b in range(B):
        in_maps.append({
            "x": x[b],
            "imp": np.ascontiguousarray(
                importance[b].reshape(N_SCHUNKS, SCHUNK).T),
            "w_proj": W_proj,
            "b_proj": b_proj.reshape(D_SPACE, 1),
            "embt": embT,
        })

    res = run_bass_kernel_spmd(nc, in_maps, list(range(N_CORES)), trace=TRACE)
    if TRACE:
        kernel.last_results = res

    w_all = np.stack([res.results[b]["wout"].reshape(-1)[:NTOT]
                      for b in range(B)])  # [8, 3840]

    e_qk = N_QK
    e_v = N_QK + N_V
    e_r = e_v + N_REL
    idx_qk = _topk_sorted_idx(w_all[:, :e_qk], TOPK_QK)
    idx_v = _topk_sorted_idx(w_all[:, e_qk:e_v], TOPK_V)
    rel = w_all[:, e_v:e_r]
    rel_Q = _topk_sparsify(rel, TOPK_REL)
    rel_K = rel_Q.copy()
    val_w = _topk_sparsify(w_all[:, e_r:], TOPK_VAL)
    return idx_qk, idx_v, rel_Q, rel_K, val_w
